# revision 14
# baseline (speedup 1.0000x reference)
"""Self-attention (SAGAN-style) Trainium2 kernel, v6.

Reference computation (per batch sample):
    theta = w_theta @ x            # [32, 4096]
    phi   = pool2x2(w_phi @ x)     # [32, 1024]
    g     = pool2x2(w_g @ x)       # [128, 1024]
    beta  = softmax(theta.T @ phi, axis=-1)   # [4096, 1024]
    attn  = g @ beta.T             # [128, 4096]
    out   = gamma * (w_o @ attn) + x

Sharding: data-parallel over batch; B=16 over 8 cores -> 2 samples/core.

Schedule: the two samples alternate at quarter granularity
(b0.qt0, b1.qt0, b0.qt1, ...), and attention/epilogue units trail their
score quarter by one full phase (~8us). Every PE instruction's inputs are
ready long before the PE reaches it, so the PE stream stays dense — the
TRN2 tensor engine only reaches its full clock after ~3us of gapless
execution, and each micro-stall drops it back to half speed.

Engine placement: exps + theta/output evacuation on ScalarE; pools
(single fused 2x2 reduce from PSUM), softmax-denominator partial sums,
reciprocal and attention normalize on DVE; x casting-loads on the GPSIMD
SWDGE queue; output stores on SP. The softmax denominator costs one
K=128 ones-matmul per chunk (broadcast over partitions); the residual
add is an identity-matmul accumulated into the out-projection PSUM
group.
"""

import numpy as np

import concourse.bacc as bacc
import concourse.mybir as mybir
from concourse import tile
from concourse.bass_utils import run_bass_kernel_spmd
from concourse.alu_op_type import AluOpType

F32 = mybir.dt.float32
BF16 = mybir.dt.bfloat16
EXP = mybir.ActivationFunctionType.Exp

B, C, H, W = 16, 256, 64, 64
N = H * W            # 4096
M = N // 4           # 1024
C8 = C // 8          # 32
C2 = C // 2          # 128
NCORES = 8
BPC = B // NCORES    # 2 samples per core
MC = M // 128        # 8 m-chunks


def build_kernel():
    nc = bacc.Bacc("TRN2", target_bir_lowering=False, debug=False)

    x_d = nc.declare_dram_parameter("x", [BPC, C, N], F32, isOutput=False)
    wq_d = nc.declare_dram_parameter("wq", [2, 128, 64], F32, isOutput=False)
    wg_d = nc.declare_dram_parameter("wg", [2, 128, C2], F32, isOutput=False)
    wo_d = nc.declare_dram_parameter("wo", [2, C2, 128], F32, isOutput=False)
    id_d = nc.declare_dram_parameter("ident", [128, 128], F32, isOutput=False)
    out_d = nc.declare_dram_parameter("out", [BPC, C, N], F32, isOutput=True)

    with tile.TileContext(nc) as tc:
        with (
            tc.tile_pool(name="const", bufs=1) as constp,
            tc.tile_pool(name="data", bufs=1) as datap,
            tc.tile_pool(name="small", bufs=3) as smallp,
            tc.tile_pool(name="outs", bufs=4) as outp,
            tc.tile_pool(name="ps_big", bufs=2, space="PSUM") as psb,
            tc.tile_pool(name="ps_u", bufs=4, space="PSUM") as psu,
        ):
            # ---- weights (casting DMAs f32->bf16 on GPSIMD SWDGE) ----
            wq, wg, wo = [], [], []
            for cc in range(2):
                t = constp.tile([128, 64], BF16, tag=f"wq{cc}", name=f"wq{cc}")
                nc.gpsimd.dma_start(t[:], wq_d[cc])
                wq.append(t)
                t = constp.tile([128, C2], BF16, tag=f"wg{cc}", name=f"wg{cc}")
                nc.gpsimd.dma_start(t[:], wg_d[cc])
                wg.append(t)
            for oc in range(2):
                t = constp.tile([C2, 128], BF16, tag=f"wo{oc}", name=f"wo{oc}")
                nc.gpsimd.dma_start(t[:], wo_d[oc])
                wo.append(t)
            id_b = constp.tile([128, 128], BF16, tag="id_b", name="id_b")
            nc.gpsimd.dma_start(id_b[:], id_d[:])
            ones = constp.tile([128, 128], BF16, tag="ones", name="ones")
            nc.gpsimd.memset(ones[:], 1.0)

            # ---- x: bf16 casting loads, resident for residual ----
            xb = [dict() for _ in range(BPC)]
            for b in range(BPC):
                for half in range(2):
                    for cc in range(2):
                        t = datap.tile([128, 2048], BF16, tag=f"xb{b}{cc}{half}",
                                       name=f"xb{b}_{cc}_{half}")
                        xb[b][(cc, half)] = t
            for b in range(BPC):
                for half in range(2):
                    for p0 in range(0, 2048, 512):
                        for cc in range(2):
                            src = slice(half * 2048 + p0, half * 2048 + p0 + 512)
                            nc.gpsimd.dma_start(
                                xb[b][(cc, half)][:, p0:p0 + 512],
                                x_d[b, cc * 128:(cc + 1) * 128, src])

            # ---- per-sample state ----
            st = []
            for b in range(BPC):
                s = dict(aps={}, at={})
                s["th2"] = datap.tile([32, N], BF16, tag=f"th2{b}",
                                      name=f"th2_{b}")
                s["ph2"] = datap.tile([32, M], BF16, tag=f"ph2{b}",
                                      name=f"ph2_{b}")
                s["gp"] = datap.tile([C2, M], BF16, tag=f"gp{b}", name=f"gp_{b}")
                # ets / psum4 are 2-quarter rings along the free axis
                s["ets"] = [datap.tile([128, 2048], BF16, tag=f"ets{b}{mc}",
                                       name=f"ets{b}_{mc}") for mc in range(MC)]
                s["psum4"] = [datap.tile([128, 2048], BF16, tag=f"ps4_{b}{j}",
                                         name=f"ps4_{b}_{j}") for j in range(4)]
                s["gts"] = [None] * MC
                st.append(s)

            # ---------------- emitters ----------------
            def proj_chunk(b, i):
                half, off = i // 4, (i % 4) * 512
                s = st[b]
                xs = [xb[b][(cc, half)][:, off:off + 512] for cc in range(2)]
                big = psb.tile([128, 1024], F32, tag="big", name=f"pj{b}_{i}")
                for cc in range(2):
                    nc.tensor.matmul(big[:, 0:512], wg[cc][:], xs[cc],
                                     start=(cc == 0), stop=(cc == 1),
                                     skip_group_check=True)
                for cc in range(2):
                    nc.tensor.matmul(big[0:64, 512:1024], wq[cc][:], xs[cc],
                                     start=(cc == 0), stop=(cc == 1),
                                     skip_group_check=True)
                sl = slice(i * 512, (i + 1) * 512)
                nc.scalar.copy(s["th2"][:, sl], big[0:32, 512:1024])
                # fused 2x2 maxpools: single DVE reduce over the two innermost
                # (pair) axes, straight out of PSUM
                msl = slice(i * 128, (i + 1) * 128)
                pv = big[32:64, 512:1024].rearrange(
                    "p (h2 hb w2 two) -> p h2 w2 hb two", h2=4, hb=2, w2=32, two=2)
                nc.vector.tensor_reduce(
                    s["ph2"][:, msl].rearrange("p (h2 w2) -> p h2 w2", h2=4, w2=32),
                    pv, mybir.AxisListType.XY, AluOpType.max)
                gv = big[:, 0:512].rearrange(
                    "p (h2 hb w2 two) -> p h2 w2 hb two", h2=4, hb=2, w2=32, two=2)
                nc.vector.tensor_reduce(
                    s["gp"][:, msl].rearrange("p (h2 w2) -> p h2 w2", h2=4, w2=32),
                    gv, mybir.AxisListType.XY, AluOpType.max)

            def tp_chunk(b, mc):
                s = st[b]
                tp = psb.tile([128, 128], BF16, tag="big", name=f"tp{b}_{mc}")
                nc.tensor.transpose(tp[:], s["gp"][:, mc * 128:(mc + 1) * 128],
                                    id_b[:])
                gt = datap.tile([128, 128], BF16, tag=f"gt{b}{mc}",
                                name=f"gt{b}_{mc}")
                nc.vector.tensor_copy(gt[:], tp[:])
                s["gts"][mc] = gt

            def score_round(b, qt, mc):
                s = st[b]
                ring = (qt % 2) * 1024
                qsl = slice(ring, ring + 1024)
                sp = psb.tile([128, 1024], F32, tag="big", name=f"sp{b}_{qt}_{mc}")
                for hf in range(2):
                    nsl = slice(qt * 1024 + hf * 512, qt * 1024 + (hf + 1) * 512)
                    nc.tensor.matmul(sp[:, hf * 512:(hf + 1) * 512],
                                     s["ph2"][:, mc * 128:(mc + 1) * 128],
                                     s["th2"][:, nsl], start=True, stop=True)
                nc.scalar.activation(s["ets"][mc][:, qsl], sp[:], EXP)
                # denominator partials: pairwise adds + in-place tree on DVE
                if mc % 2 == 1:
                    p = mc // 2
                    ps4 = s["psum4"]
                    nc.vector.tensor_tensor(ps4[p][:, qsl],
                                            s["ets"][mc - 1][:, qsl],
                                            s["ets"][mc][:, qsl], AluOpType.add)
                    if mc == 7:
                        nc.vector.tensor_tensor(ps4[0][:, qsl], ps4[0][:, qsl],
                                                ps4[1][:, qsl], AluOpType.add)
                        nc.vector.tensor_tensor(ps4[2][:, qsl], ps4[2][:, qsl],
                                                ps4[3][:, qsl], AluOpType.add)
                        nc.vector.tensor_tensor(ps4[0][:, qsl], ps4[0][:, qsl],
                                                ps4[2][:, qsl], AluOpType.add)

            def unit_attn(b, i):
                s = st[b]
                ring = ((i // 2) % 2) * 1024 + (i % 2) * 512
                rsl = slice(ring, ring + 512)
                aps = psu.tile([128, 512], F32, tag="u", name=f"aps{b}_{i}")
                s["aps"][i] = aps
                for mc in range(MC):
                    nc.tensor.matmul(aps[:], s["gts"][mc][:],
                                     s["ets"][mc][:, rsl],
                                     start=(mc == 0), stop=(mc == MC - 1),
                                     skip_group_check=True)

            def unit_den(b, i):
                s = st[b]
                ring = ((i // 2) % 2) * 1024 + (i % 2) * 512
                rsl = slice(ring, ring + 512)
                dps = psu.tile([128, 512], F32, tag="u", name=f"dps{b}_{i}")
                nc.tensor.matmul(dps[:], ones[:], s["psum4"][0][:, rsl],
                                 start=True, stop=True)
                rec = smallp.tile([128, 512], F32, tag="rec", name=f"rec{b}_{i}")
                nc.vector.reciprocal_approx_fast(rec[:], dps[:])
                at = smallp.tile([128, 512], BF16, tag="at", name=f"at{b}_{i}")
                nc.vector.scalar_tensor_tensor(
                    at[:], s["aps"][i][:], 1.0, rec[:],
                    AluOpType.bypass, AluOpType.mult)
                s["at"][i] = at

            def unit_out(b, i):
                s = st[b]
                nsl = slice(i * 512, (i + 1) * 512)
                half, off = i // 4, (i % 4) * 512
                for oc in range(2):
                    op = psu.tile([128, 512], F32, tag="u", name=f"op{b}_{i}_{oc}")
                    nc.tensor.matmul(op[:], id_b[:],
                                     xb[b][(oc, half)][:, off:off + 512],
                                     start=True, stop=False,
                                     skip_group_check=True)
                    nc.tensor.matmul(op[:], wo[oc][:], s["at"][i][:],
                                     start=False, stop=True,
                                     skip_group_check=True)
                    osb = outp.tile([128, 512], F32, tag="osb",
                                    name=f"osb{b}_{i}_{oc}")
                    nc.scalar.copy(osb[:], op[:])
                    nc.sync.dma_start(out_d[b, oc * 128:(oc + 1) * 128, nsl],
                                      osb[:])

            # ---------------- emission schedule ----------------
            pending = []

            def pop(k):
                for _ in range(k):
                    if pending:
                        pending.pop(0)()

            for b in range(BPC):
                for i in range(MC):
                    proj_chunk(b, i)
                for mc in range(MC):
                    tp_chunk(b, mc)

            for qt in range(4):
                for b in range(BPC):
                    for mc in range(MC):
                        score_round(b, qt, mc)
                        if 1 <= mc <= 6:
                            pop(1)
                    # queue this quarter's units; they pop one phase later
                    i0, i1 = 2 * qt, 2 * qt + 1
                    pending.append(lambda bb=b, i=i0: unit_attn(bb, i))
                    pending.append(lambda bb=b, i=i0: unit_den(bb, i))
                    pending.append(lambda bb=b, i=i1: unit_attn(bb, i))
                    pending.append(lambda bb=b, i=i1: unit_den(bb, i))
                    pending.append(lambda bb=b, i=i0: unit_out(bb, i))
                    pending.append(lambda bb=b, i=i1: unit_out(bb, i))
            while pending:
                pop(1)

    nc.compile()
    return nc


_NC_CACHE = None


def _get_nc():
    global _NC_CACHE
    if _NC_CACHE is None:
        _NC_CACHE = build_kernel()
    return _NC_CACHE


def prep_inputs(x, w_theta, w_phi, w_g, w_o, gamma):
    """Host-side prep: shard x over 8 cores; transpose/scale/pack weights."""
    x = np.asarray(x, dtype=np.float32).reshape(B, C, N)
    w_theta = np.asarray(w_theta, dtype=np.float32)
    w_phi = np.asarray(w_phi, dtype=np.float32)
    w_g = np.asarray(w_g, dtype=np.float32)
    w_o = np.asarray(w_o, dtype=np.float32)
    gamma = np.float32(gamma)

    wqT = np.concatenate([w_theta.T, w_phi.T], axis=1)       # [256, 64]
    wq = np.ascontiguousarray(wqT.reshape(2, 128, 64))
    wgq = np.ascontiguousarray(w_g.T.reshape(2, 128, C2))
    woT = (gamma * w_o).T                                     # [128, 256]
    wo = np.ascontiguousarray(woT.reshape(C2, 2, 128).transpose(1, 0, 2))
    ident = np.eye(128, dtype=np.float32)

    in_maps = []
    for core in range(NCORES):
        shard = np.ascontiguousarray(x[core * BPC:(core + 1) * BPC])
        in_maps.append({"x": shard, "wq": wq, "wg": wgq, "wo": wo,
                        "ident": ident})
    return in_maps


def run(inputs, trace=False, **kw):
    nc = _get_nc()
    in_maps = prep_inputs(**inputs)
    res = run_bass_kernel_spmd(nc, in_maps, core_ids=list(range(NCORES)),
                               trace=trace, **kw)
    outs = [res.results[i]["out"] for i in range(NCORES)]
    full = np.concatenate(outs, axis=0).reshape(B, C, H, W).astype(np.float32)
    return full, res


def kernel(**inputs):
    full, _ = run(inputs, trace=False)
    return full
